# revision 40
# baseline (speedup 1.0000x reference)
"""3-layer GCN (GCNConv x3, PyG defaults) on 8 Trainium2 NeuronCores.

Strategy (graph/data parallel, v2):
  - Nodes sharded 8 ways by destination range (6250/core, padded to 6272).
    Per core, nodes are sorted by total in-degree (incl. self-loop) desc;
    slot k of the segment-sum covers the prefix of nodes with degree > k.
  - Per layer: dense X@W on the TensorEngine (activations kept as
    [token, feat] per 128-token rank), scaled by deg^-1/2 into p = dinv*h,
    one AllGather collective puts every core's p into a DRAM buffer AG
    laid out as 50176 contiguous 256 B tokens.
  - Aggregation gathers each edge's source token DIRECTLY FROM DRAM with
    gpsimd dma_gather (transposed output [128 feat, edges]) and segment-sums
    on the TensorEngine as identity-matmul PSUM accumulation, slot-major.
    A SINGLE pass covers all 50176 source tokens: int16 gather indices are
    signed, so the gather base is placed at token 25088 and indices span
    [-25088, 25087]. The SWDGE ucode trims trailing NEGATIVE indices, so
    every gather chunk is padded with a positive zero-data token.
  - All deg^-1/2 scales fold into per-partition scalars of PSUM-drain
    activations (relu(d*x) = d*relu(x) for d>0).
All 8 cores run one identical program; only input data differs per core.
"""
import sys

sys.path.insert(0, "/opt/trn_rl_repo")

import numpy as np
import ml_dtypes

from concourse import bass, bacc, mybir
from concourse import tile
from concourse.bass_utils import run_bass_kernel_spmd

BF16 = ml_dtypes.bfloat16
C = 8
BLK = 512
CHUNK = 4096
FOUT_PAD = 128  # W_out columns padded so layer-3 tokens share the 256B layout


class Plan:
    pass


def _pack_idx(vals):
    """int16 array -> [128, len/16] wrapped (i -> [i%16, i//16]) replicated x8."""
    n = len(vals)
    assert n % 16 == 0
    a = np.asarray(vals, dtype=np.int16).reshape(n // 16, 16).T  # [16, n/16]
    return np.tile(a, (8, 1))


def preprocess(x, edge_index, n_nodes):
    p = Plan()
    N = n_nodes
    assert N % C == 0
    npc = N // C
    sec = (npc + 127) // 128 * 128
    p.npc, p.sec = npc, sec
    p.nranks = sec // 128
    p.nblk = (npc + BLK - 1) // BLK
    p.fin = x.shape[1]
    assert p.fin % 128 == 0
    p.finc = p.fin // 128
    ntok = C * sec
    half = ntok // 2
    p.half = half
    # positive zero-data pad token: last pad position of core C-1
    pad_pos = sec - 1
    PADTOK = ((C - 1) * 128 + pad_pos % 128) * p.nranks + pad_pos // 128
    assert PADTOK >= half

    src = np.asarray(edge_index[0], dtype=np.int64)
    dst = np.asarray(edge_index[1], dtype=np.int64)
    indeg = np.bincount(dst, minlength=N).astype(np.int64)  # real in-edges
    dinv = (1.0 / np.sqrt((indeg + 1).astype(np.float32))).astype(np.float32)

    # per-core node order: in-degree desc; position within core
    pos_g = np.empty(N, np.int64)
    perm = np.empty((C, npc), np.int64)
    for m in range(C):
        pl = np.argsort(-indeg[m * npc:(m + 1) * npc], kind="stable")
        perm[m] = pl
        pos_g[m * npc + pl] = np.arange(npc)
    p.perm = perm
    # DRAM token id of each node: t = (core*128 + pos%128)*nranks + pos//128
    tok = ((np.arange(N) // npc) * 128 + pos_g % 128) * p.nranks + pos_g // 128

    # slot-major cell structure over real edges (self term is added on-chip)
    d_core = dst // npc
    all_cells = [None] * C
    all_data = [None] * C
    for m in range(C):
        sel = d_core == m
        jp = pos_g[dst[sel]]
        ts = tok[src[sel]]
        cnt = np.bincount(jp, minlength=npc)
        order = np.argsort(jp, kind="stable")
        js, tss = jp[order], ts[order]
        starts = np.zeros(npc, np.int64)
        starts[1:] = np.cumsum(cnt)[:-1]
        kk = np.arange(len(js)) - starts[js]
        bb = js // BLK
        o2 = np.lexsort((js, kk, bb))
        js, tss, kk, bb = js[o2], tss[o2], kk[o2], bb[o2]
        cells = {}
        data = {}
        cell_id = bb * 8192 + kk
        uniq, first = np.unique(cell_id, return_index=True)
        bounds = list(first) + [len(cell_id)]
        for i, u in enumerate(uniq):
            b, k = int(u) // 8192, int(u) % 8192
            seg = slice(bounds[i], bounds[i + 1])
            cells[(b, k)] = bounds[i + 1] - bounds[i]
            data[(b, k)] = tss[seg]
        all_cells[m] = cells
        all_data[m] = data

    # uniform geometry across cores: per-cell max width and column offset
    cells_max = {}
    for m in range(C):
        for key, n in all_cells[m].items():
            cells_max[key] = max(cells_max.get(key, 0), n)
    cell_col = {}  # (b,k) -> first covered PSUM column (cells cover a prefix)
    for (b, k), n in cells_max.items():
        cell_col[(b, k)] = 0

    # pack cells (b-major, k asc) into chunks; cells may split across chunks;
    # every chunk reserves >=1 trailing positive pad column (negative-trim
    # workaround), and lengths are multiples of 128
    # block processing order: rotate the last 3 blocks to the front so the
    # per-layer tail (last block -> dense -> allgather write) is short
    p.border = list(range(p.nblk - 3, p.nblk)) + list(range(p.nblk - 3))
    chunks = []
    pieces = {}  # (b,k) -> list of (chunk_idx, off_in_chunk, ncols, col0)
    cur = CHUNK  # force new chunk
    cap = 2048  # mid-size first chunk shortens each post-collective ramp
    for b in p.border:
        ks = sorted(k for (bb2, k) in cells_max if bb2 == b)
        for k in ks:
            n = cells_max[(b, k)]
            col0 = 0
            pl = []
            while n > 0:
                if cur >= cap - 1:
                    cap = 2048 if not chunks else CHUNK
                    chunks.append(0)
                    cur = 0
                t = min(n, cap - 1 - cur)
                pl.append((len(chunks) - 1, cur, t, col0))
                cur += t
                chunks[-1] = cur
                col0 += t
                n -= t
            pieces[(b, k)] = pl
    chunks = [l + 128 - l % 128 if l % 128 else l + 128 for l in chunks]
    p.chunks = chunks
    p.pieces = pieces
    total = sum(chunks)
    coff = np.concatenate([[0], np.cumsum(chunks)])

    p.idx = []
    for m in range(C):
        buf = np.full(total, PADTOK, np.int64)
        for key, pl in pieces.items():
            d = all_data[m].get(key)
            if d is None:
                continue
            for (ci, off, t, col0) in pl:
                seg = d[col0:col0 + t]
                buf[coff[ci] + off: coff[ci] + off + len(seg)] = seg
        p.idx.append(_pack_idx((buf - half).astype(np.int16)))

    # per-core dense inputs; d1 (= deg^-1/2 per token) is folded into x so the
    # layer-1 dense matmul directly produces p1 = d1 * (x @ W1)
    p.xT = []
    p.d1 = []
    p.d2 = []
    for m in range(C):
        pl = perm[m]
        xm = np.asarray(x[m * npc:(m + 1) * npc][pl], dtype=np.float32)
        xm = xm * dinv[m * npc + pl][:, None]
        xt = np.zeros((p.nranks, 128, p.finc, 128), BF16)
        flat = xt.reshape(sec, p.finc, 128)
        flat[:npc] = xm.reshape(npc, p.finc, 128).astype(BF16)
        p.xT.append(np.ascontiguousarray(xt.transpose(1, 0, 2, 3)))
        dv = np.zeros(sec, np.float32)
        dv[:npc] = dinv[m * npc + pl]
        dvt = dv.reshape(p.nranks, 128).T.copy()  # [128, nranks]
        p.d1.append(dvt)
        p.d2.append(dvt * dvt)
    return p


# --------------------------------------------------------------------------
# Device kernel builder (one program, SPMD across 8 cores).
# --------------------------------------------------------------------------
def build_kernel(p, fmid, fout, bias_path=False, debug_stage=99, sim_mode=False):
    dt = mybir.dt
    nc = bacc.Bacc("TRN2", num_swdge_queues=1)
    sec, nranks, npc, nblk = p.sec, p.nranks, p.npc, p.nblk
    ntok = C * sec
    ltot = sum(p.chunks)

    xT_d = nc.dram_tensor("xT", [128, nranks, p.finc, 128], dt.bfloat16, kind="ExternalInput")
    w1_d = nc.dram_tensor("w1", [128, p.finc, fmid], dt.bfloat16, kind="ExternalInput")
    w2_d = nc.dram_tensor("w2", [fmid, fmid], dt.bfloat16, kind="ExternalInput")
    w3_d = nc.dram_tensor("w3", [fmid, FOUT_PAD], dt.bfloat16, kind="ExternalInput")
    d1_d = nc.dram_tensor("d1", [128, nranks], dt.float32, kind="ExternalInput")
    d2_d = nc.dram_tensor("d2", [128, nranks], dt.float32, kind="ExternalInput")
    id_d = nc.dram_tensor("ident", [128, 128], dt.bfloat16, kind="ExternalInput")
    il_d = nc.dram_tensor("idx", [128, ltot // 16], dt.int16, kind="ExternalInput")
    out_d = nc.dram_tensor("out", [128, nranks * fout], dt.float32, kind="ExternalOutput")

    ag_in = nc.dram_tensor("ag_in", [128, sec], dt.bfloat16)
    AG = [nc.dram_tensor(f"ag_out{i}", [C * 128, sec], dt.bfloat16, addr_space="Shared")
          for i in range(2)]

    # kmax per block
    kmax = {}
    for (b, k) in p.pieces:
        kmax[b] = max(kmax.get(b, -1), k)
    coff = [0]
    for l in p.chunks:
        coff.append(coff[-1] + l)

    with tile.TileContext(nc) as tc:
        with (
            tc.tile_pool(name="main", bufs=1) as main,
            tc.tile_pool(name="mp", bufs=6) as mp,
            tc.tile_pool(name="xc", bufs=3) as xcp,
            tc.tile_pool(name="psb", bufs=3, space=bass.MemorySpace.PSUM) as psb,
            tc.tile_pool(name="ppp", bufs=3, space=bass.MemorySpace.PSUM) as ppp,
            tc.tile_pool(name="ptp", bufs=2, space=bass.MemorySpace.PSUM) as ptp,
        ):
            sA = main.tile([128, sec], dt.bfloat16)
            pself = main.tile([128, sec], dt.bfloat16)
            work = main.tile([128, 1, sec], dt.bfloat16)
            ident = main.tile([128, 128], dt.bfloat16)
            w1 = main.tile([128, p.finc, fmid], dt.bfloat16)
            w2 = main.tile([fmid, fmid], dt.bfloat16)
            w3 = main.tile([fmid, FOUT_PAD], dt.bfloat16)
            d1 = main.tile([128, nranks], dt.float32)
            d2 = main.tile([128, nranks], dt.float32)
            il = main.tile([128, ltot // 16], dt.int16)

            # load order follows first use: L1 dense needs ident/w1 (+xT,
            # issued in pstage_l1); idx/w2/w3/d's are needed only later
            nc.sync.dma_start(ident[:], id_d[:])
            nc.sync.dma_start(w1[:], w1_d[:])

            # rank groups for split allgather-input writes; group g (4 ranks)
            # is ready as soon as block g has been relu'd
            rgroups = [(4 * g, min(4 * g + 4, nranks), g) for g in range(nblk)]
            NG = len(rgroups)

            def ag_part(g):
                a, b, _ = rgroups[g]
                nc.sync.dma_start(ag_in[:, a * 128:b * 128],
                                  work[:, 0, a * 128:b * 128])

            def ag_finish(buf):
                if sim_mode:
                    # stand-in for the collective: tiny DMA bridging the
                    # ag_in -> AG dependency so the timeline stays ordered
                    nc.sync.dma_start(AG[buf][C * 64: C * 64 + 1, :], ag_in[0:1, :])
                else:
                    nc.gpsimd.collective_compute(
                        "AllGather", mybir.AluOpType.bypass,
                        replica_groups=[list(range(C))],
                        ins=[ag_in.ap().opt()], outs=[AG[buf].ap().opt()])

            def pself_group(a, b):
                # feature-major copy of ranks [a,b)'s fresh p-values (work is
                # token-major); used as the self-loop term of the next layer
                n = b - a
                pt = ptp.tile([128, 4, 128], dt.bfloat16, tag="pt")
                for i, c in enumerate(range(a, b)):
                    nc.tensor.transpose(pt[:, i, :],
                                        work[:, 0, c * 128:(c + 1) * 128],
                                        ident[:])
                nc.scalar.activation(
                    pself[:, a * 128:b * 128],
                    pt[:, 0:n, :].rearrange("q c f -> q (c f)"),
                    mybir.ActivationFunctionType.Copy)

            def pstage_group(a, b, w, scale):
                n = b - a
                ps = ppp.tile([128, 4, 128], dt.float32, tag="pp")
                for i, c in enumerate(range(a, b)):
                    nc.tensor.matmul(ps[:, i, 0:w.shape[-1]],
                                     work[:, 0, c * 128:(c + 1) * 128], w[:],
                                     start=True, stop=True)
                nc.vector.tensor_tensor(
                    work[:, 0, a * 128:b * 128].rearrange("q (c f) -> q c f", f=128),
                    ps[:, 0:n, :],
                    scale[:, a:b].rearrange("q c -> q c ()").broadcast_to(
                        [128, n, 128]),
                    mybir.AluOpType.mult)
                pself_group(a, b)

            def pstage_l1(buf):
                GRP = 13
                ngrp = (nranks + GRP - 1) // GRP
                xts = []
                for g in range(ngrp):
                    a, b = g * GRP, min(nranks, (g + 1) * GRP)
                    xt = mp.tile([128, b - a, p.finc, 128], dt.bfloat16, tag="x")
                    nc.sync.dma_start(xt[:], xT_d[:, a:b, :, :])
                    xts.append((a, xt))
                # remaining constants, after xT on the DMA queue
                nc.sync.dma_start(d1[:], d1_d[:])
                nc.sync.dma_start(d2[:], d2_d[:])
                nc.sync.dma_start(w2[:], w2_d[:])
                nc.sync.dma_start(w3[:], w3_d[:])
                nc.sync.dma_start(il[:], il_d[:])
                nc.vector.memset(sA[:], 0.0)
                nc.vector.memset(work[:], 0.0)
                for g in range(NG):
                    ga, gb, _ = rgroups[g]
                    ps = ppp.tile([128, 4, 128], dt.float32, tag="pp")
                    for i, c in enumerate(range(ga, gb)):
                        xa, xt = xts[c // GRP]
                        for f in range(p.finc):
                            nc.tensor.matmul(ps[:, i, :], xt[:, c - xa, f, :],
                                             w1[:, f, :],
                                             start=(f == 0), stop=(f == p.finc - 1))
                    nc.vector.tensor_copy(
                        work[:, 0, ga * 128:gb * 128].rearrange(
                            "q (c f) -> q c f", f=128),
                        ps[:, 0:gb - ga, :])
                    ag_part(g)
                    pself_group(ga, gb)
                ag_finish(buf)

            def out_group(a, b, sm, et, lg):
                n = b - a
                pt = ptp.tile([128, 4, 128], dt.bfloat16, tag="pt")
                for i, c in enumerate(range(a, b)):
                    nc.tensor.transpose(pt[:, i, :], sA[:, c * 128:(c + 1) * 128],
                                        ident[:])
                nc.vector.tensor_tensor(
                    sm[:, a:b, :], pt[:, 0:n, 0:fout],
                    d1[:, a:b].rearrange("q c -> q c ()").broadcast_to(
                        [128, n, fout]),
                    mybir.AluOpType.mult)
                nc.scalar.activation(et[:, a:b, :], sm[:, a:b, :],
                                     mybir.ActivationFunctionType.Exp)
                nc.vector.reduce_sum(lg[:, a:b], et[:, a:b, :],
                                     axis=mybir.AxisListType.X)

            def aggregate(layer):
                """Gather + identity-matmul segment-sum; per block: drain into
                sA, then immediately run whatever downstream work that block
                unblocks (relu + dense + ag write, or the output ranks)."""
                buf = layer % 2
                in_ap = AG[buf].ap().rearrange(
                    "a (r f) -> (a r) f", f=128)[p.half:, :]
                mts = {}
                for ci, clen in enumerate(p.chunks):
                    m = mp.tile([128, 1, clen], dt.bfloat16, tag="m")
                    nc.gpsimd.dma_gather(
                        out_ap=m[:], in_ap=in_ap,
                        idxs_ap=il[:, coff[ci] // 16:(coff[ci] + clen) // 16],
                        num_idxs=clen, num_idxs_reg=clen, elem_size=128,
                        transpose=True, single_packet=False, queue_num=0)
                    mts[ci] = m
                last = layer == 2
                if last:
                    sm = mp.tile([128, nranks, fout], dt.float32, tag="o")
                    et = mp.tile([128, nranks, fout], dt.float32, tag="o")
                    lg = xcp.tile([128, nranks], dt.float32, tag="xc")
                    w_next = scale = None
                else:
                    w_next = w2 if layer == 0 else w3
                    scale = d2
                for b in p.border:
                    bsz = min(BLK, npc - b * BLK)
                    a0 = b * BLK
                    ps = psb.tile([128, BLK], dt.float32, tag="ps")
                    # self-loop term opens the accumulation group
                    nc.tensor.matmul(ps[:, 0:bsz], ident[:], pself[:, a0:a0 + bsz],
                                     start=True, stop=False)
                    plist = [(k, pc) for k in range(kmax[b] + 1)
                             for pc in p.pieces[(b, k)]]
                    for i, (k, (ci, off, t, col0)) in enumerate(plist):
                        nc.tensor.matmul(ps[:, col0:col0 + t], ident[:],
                                         mts[ci][:, 0, off:off + t],
                                         start=False, stop=(i == len(plist) - 1))
                    nc.vector.tensor_copy(sA[:, a0:a0 + bsz], ps[:, 0:bsz])
                    if last:
                        out_group(4 * b, min(4 * b + 4, nranks), sm, et, lg)
                    else:
                        a1 = min(npc, (b + 1) * BLK)
                        nc.scalar.activation(work[:, 0, a0:a1], sA[:, a0:a1],
                                             mybir.ActivationFunctionType.Relu)
                        pstage_group(rgroups[b][0], rgroups[b][1], w_next, scale)
                        ag_part(b)
                if last:
                    nc.scalar.activation(lg[:], lg[:],
                                         mybir.ActivationFunctionType.Ln)
                    qs = [0, 12, 24, 36, nranks]
                    for i in range(4):
                        a, b = qs[i], qs[i + 1]
                        nc.vector.tensor_tensor(
                            sm[:, a:b, :], sm[:, a:b, :],
                            lg[:, a:b].rearrange("q c -> q c ()").broadcast_to(
                                [128, b - a, fout]),
                            mybir.AluOpType.subtract)
                        nc.sync.dma_start(
                            out_d[:, a * fout:b * fout],
                            sm[:, a:b, :].rearrange("q c f -> q (c f)"))
                else:
                    ag_finish((layer + 1) % 2)

            # ---- program ----
            pstage_l1(0)
            for layer in range(3):
                aggregate(layer)
    nc.compile()
    return nc


# --------------------------------------------------------------------------
# Entry point
# --------------------------------------------------------------------------
def _make_in_maps(p, inputs, fmid, fout):
    W_in = np.asarray(inputs["W_in"], dtype=np.float32)
    W_mid = np.asarray(inputs["W_mid"], dtype=np.float32)
    W_out = np.asarray(inputs["W_out"], dtype=np.float32)
    w1 = np.ascontiguousarray(
        W_in.reshape(p.finc, 128, fmid).transpose(1, 0, 2).astype(BF16))
    w2 = np.ascontiguousarray(W_mid.astype(BF16))
    w3 = np.zeros((fmid, FOUT_PAD), BF16)
    w3[:, :fout] = W_out.astype(BF16)
    ident = np.eye(128, dtype=np.float32).astype(BF16)
    in_maps = []
    for m in range(C):
        in_maps.append({
            "xT": p.xT[m].reshape(128, p.nranks, p.finc, 128),
            "w1": w1, "w2": w2, "w3": w3,
            "d1": p.d1[m], "d2": p.d2[m], "ident": ident,
            "idx": p.idx[m],
        })
    return in_maps


def _run(inputs, trace=False, trace_cores=None, debug_stage=99):
    x = np.asarray(inputs["x"], dtype=np.float32)
    edge_index = np.asarray(inputs["edge_index"])
    W_in = np.asarray(inputs["W_in"], dtype=np.float32)
    W_out = np.asarray(inputs["W_out"], dtype=np.float32)
    for bname in ("b_in", "b_mid", "b_out"):
        if np.any(np.asarray(inputs[bname])):
            raise NotImplementedError("nonzero bias path not implemented")

    N, fin = x.shape
    fmid = W_in.shape[1]
    fout = W_out.shape[1]
    p = preprocess(x, edge_index, N)

    nc = build_kernel(p, fmid, fout)

    in_maps = _make_in_maps(p, inputs, fmid, fout)
    kw = {}
    if trace:
        kw = dict(trace=True, trace_cores=trace_cores or [0])
    r = run_bass_kernel_spmd(nc, in_maps, core_ids=list(range(C)), **kw)

    out = np.empty((N, fout), np.float32)
    for m in range(C):
        res = r.results[m]["out"]  # [128, nranks*fout] partition-major
        rows = res.reshape(128, p.nranks, fout).transpose(1, 0, 2).reshape(p.sec, fout)
        out[m * p.npc + p.perm[m]] = rows[:p.npc]
    return out, r


def kernel(**inputs) -> np.ndarray:
    out, _ = _run(inputs)
    return out


# revision 42
# speedup vs baseline: 1.0069x; 1.0069x over previous
"""3-layer GCN (GCNConv x3, PyG defaults) on 8 Trainium2 NeuronCores.

Strategy (graph/data parallel, v2):
  - Nodes sharded 8 ways by destination range (6250/core, padded to 6272).
    Per core, nodes are sorted by total in-degree (incl. self-loop) desc;
    slot k of the segment-sum covers the prefix of nodes with degree > k.
  - Per layer: dense X@W on the TensorEngine (activations kept as
    [token, feat] per 128-token rank), scaled by deg^-1/2 into p = dinv*h,
    one AllGather collective puts every core's p into a DRAM buffer AG
    laid out as 50176 contiguous 256 B tokens.
  - Aggregation gathers each edge's source token DIRECTLY FROM DRAM with
    gpsimd dma_gather (transposed output [128 feat, edges]) and segment-sums
    on the TensorEngine as identity-matmul PSUM accumulation, slot-major.
    A SINGLE pass covers all 50176 source tokens: int16 gather indices are
    signed, so the gather base is placed at token 25088 and indices span
    [-25088, 25087]. The SWDGE ucode trims trailing NEGATIVE indices, so
    every gather chunk is padded with a positive zero-data token.
  - All deg^-1/2 scales fold into per-partition scalars of PSUM-drain
    activations (relu(d*x) = d*relu(x) for d>0).
All 8 cores run one identical program; only input data differs per core.
"""
import sys

sys.path.insert(0, "/opt/trn_rl_repo")

import numpy as np
import ml_dtypes

from concourse import bass, bacc, mybir
from concourse import tile
from concourse.bass_utils import run_bass_kernel_spmd

BF16 = ml_dtypes.bfloat16
C = 8
BLK = 512
CHUNK = 4096
FOUT_PAD = 128  # W_out columns padded so layer-3 tokens share the 256B layout


class Plan:
    pass


def _pack_idx(vals):
    """int16 array -> [128, len/16] wrapped (i -> [i%16, i//16]) replicated x8."""
    n = len(vals)
    assert n % 16 == 0
    a = np.asarray(vals, dtype=np.int16).reshape(n // 16, 16).T  # [16, n/16]
    return np.tile(a, (8, 1))


def preprocess(x, edge_index, n_nodes):
    p = Plan()
    N = n_nodes
    assert N % C == 0
    npc = N // C
    sec = (npc + 127) // 128 * 128
    p.npc, p.sec = npc, sec
    p.nranks = sec // 128
    p.nblk = (npc + BLK - 1) // BLK
    p.fin = x.shape[1]
    assert p.fin % 128 == 0
    p.finc = p.fin // 128
    ntok = C * sec
    half = ntok // 2
    p.half = half
    # positive zero-data pad token: last pad position of core C-1
    pad_pos = sec - 1
    PADTOK = ((C - 1) * 128 + pad_pos % 128) * p.nranks + pad_pos // 128
    assert PADTOK >= half

    src = np.asarray(edge_index[0], dtype=np.int64)
    dst = np.asarray(edge_index[1], dtype=np.int64)
    indeg = np.bincount(dst, minlength=N).astype(np.int64)  # real in-edges
    dinv = (1.0 / np.sqrt((indeg + 1).astype(np.float32))).astype(np.float32)

    # per-core node order: in-degree desc; position within core
    pos_g = np.empty(N, np.int64)
    perm = np.empty((C, npc), np.int64)
    for m in range(C):
        pl = np.argsort(-indeg[m * npc:(m + 1) * npc], kind="stable")
        perm[m] = pl
        pos_g[m * npc + pl] = np.arange(npc)
    p.perm = perm
    # DRAM token id of each node: t = (core*128 + pos%128)*nranks + pos//128
    tok = ((np.arange(N) // npc) * 128 + pos_g % 128) * p.nranks + pos_g // 128

    # slot-major cell structure over real edges (self term is added on-chip)
    d_core = dst // npc
    all_cells = [None] * C
    all_data = [None] * C
    for m in range(C):
        sel = d_core == m
        jp = pos_g[dst[sel]]
        ts = tok[src[sel]]
        cnt = np.bincount(jp, minlength=npc)
        order = np.argsort(jp, kind="stable")
        js, tss = jp[order], ts[order]
        starts = np.zeros(npc, np.int64)
        starts[1:] = np.cumsum(cnt)[:-1]
        kk = np.arange(len(js)) - starts[js]
        bb = js // BLK
        o2 = np.lexsort((js, kk, bb))
        js, tss, kk, bb = js[o2], tss[o2], kk[o2], bb[o2]
        cells = {}
        data = {}
        cell_id = bb * 8192 + kk
        uniq, first = np.unique(cell_id, return_index=True)
        bounds = list(first) + [len(cell_id)]
        for i, u in enumerate(uniq):
            b, k = int(u) // 8192, int(u) % 8192
            seg = slice(bounds[i], bounds[i + 1])
            cells[(b, k)] = bounds[i + 1] - bounds[i]
            data[(b, k)] = tss[seg]
        all_cells[m] = cells
        all_data[m] = data

    # uniform geometry across cores: per-cell max width and column offset
    cells_max = {}
    for m in range(C):
        for key, n in all_cells[m].items():
            cells_max[key] = max(cells_max.get(key, 0), n)
    cell_col = {}  # (b,k) -> first covered PSUM column (cells cover a prefix)
    for (b, k), n in cells_max.items():
        cell_col[(b, k)] = 0

    # pack cells (b-major, k asc) into chunks; cells may split across chunks;
    # every chunk reserves >=1 trailing positive pad column (negative-trim
    # workaround), and lengths are multiples of 128
    # block processing order: heavy late blocks first, the tiny last block
    # (106 cols, single-rank dense group) at the very end so the per-layer
    # tail (last block -> dense -> allgather write) is minimal
    p.border = [9, 10, 11] + list(range(9)) + [p.nblk - 1]
    chunks = []
    pieces = {}  # (b,k) -> list of (chunk_idx, off_in_chunk, ncols, col0)
    cur = CHUNK  # force new chunk
    for b in p.border:
        ks = sorted(k for (bb2, k) in cells_max if bb2 == b)
        for k in ks:
            n = cells_max[(b, k)]
            col0 = 0
            pl = []
            while n > 0:
                if cur >= CHUNK - 1:
                    chunks.append(0)
                    cur = 0
                t = min(n, CHUNK - 1 - cur)
                pl.append((len(chunks) - 1, cur, t, col0))
                cur += t
                chunks[-1] = cur
                col0 += t
                n -= t
            pieces[(b, k)] = pl
    chunks = [l + 128 - l % 128 if l % 128 else l + 128 for l in chunks]
    p.chunks = chunks
    p.pieces = pieces
    total = sum(chunks)
    coff = np.concatenate([[0], np.cumsum(chunks)])

    p.idx = []
    for m in range(C):
        buf = np.full(total, PADTOK, np.int64)
        for key, pl in pieces.items():
            d = all_data[m].get(key)
            if d is None:
                continue
            for (ci, off, t, col0) in pl:
                seg = d[col0:col0 + t]
                buf[coff[ci] + off: coff[ci] + off + len(seg)] = seg
        p.idx.append(_pack_idx((buf - half).astype(np.int16)))

    # per-core dense inputs; d1 (= deg^-1/2 per token) is folded into x so the
    # layer-1 dense matmul directly produces p1 = d1 * (x @ W1)
    p.xT = []
    p.d1 = []
    p.d2 = []
    for m in range(C):
        pl = perm[m]
        xm = np.asarray(x[m * npc:(m + 1) * npc][pl], dtype=np.float32)
        xm = xm * dinv[m * npc + pl][:, None]
        xt = np.zeros((p.nranks, 128, p.finc, 128), BF16)
        flat = xt.reshape(sec, p.finc, 128)
        flat[:npc] = xm.reshape(npc, p.finc, 128).astype(BF16)
        p.xT.append(np.ascontiguousarray(xt.transpose(1, 0, 2, 3)))
        dv = np.zeros(sec, np.float32)
        dv[:npc] = dinv[m * npc + pl]
        dvt = dv.reshape(p.nranks, 128).T.copy()  # [128, nranks]
        p.d1.append(dvt)
        p.d2.append(dvt * dvt)
    return p


# --------------------------------------------------------------------------
# Device kernel builder (one program, SPMD across 8 cores).
# --------------------------------------------------------------------------
def build_kernel(p, fmid, fout, bias_path=False, debug_stage=99, sim_mode=False):
    dt = mybir.dt
    nc = bacc.Bacc("TRN2", num_swdge_queues=1)
    sec, nranks, npc, nblk = p.sec, p.nranks, p.npc, p.nblk
    ntok = C * sec
    ltot = sum(p.chunks)

    xT_d = nc.dram_tensor("xT", [128, nranks, p.finc, 128], dt.bfloat16, kind="ExternalInput")
    w1_d = nc.dram_tensor("w1", [128, p.finc, fmid], dt.bfloat16, kind="ExternalInput")
    w2_d = nc.dram_tensor("w2", [fmid, fmid], dt.bfloat16, kind="ExternalInput")
    w3_d = nc.dram_tensor("w3", [fmid, FOUT_PAD], dt.bfloat16, kind="ExternalInput")
    d1_d = nc.dram_tensor("d1", [128, nranks], dt.float32, kind="ExternalInput")
    d2_d = nc.dram_tensor("d2", [128, nranks], dt.float32, kind="ExternalInput")
    id_d = nc.dram_tensor("ident", [128, 128], dt.bfloat16, kind="ExternalInput")
    il_d = nc.dram_tensor("idx", [128, ltot // 16], dt.int16, kind="ExternalInput")
    out_d = nc.dram_tensor("out", [128, nranks * fout], dt.float32, kind="ExternalOutput")

    ag_in = nc.dram_tensor("ag_in", [128, sec], dt.bfloat16)
    AG = [nc.dram_tensor(f"ag_out{i}", [C * 128, sec], dt.bfloat16, addr_space="Shared")
          for i in range(2)]

    # kmax per block
    kmax = {}
    for (b, k) in p.pieces:
        kmax[b] = max(kmax.get(b, -1), k)
    coff = [0]
    for l in p.chunks:
        coff.append(coff[-1] + l)

    with tile.TileContext(nc) as tc:
        with (
            tc.tile_pool(name="main", bufs=1) as main,
            tc.tile_pool(name="mp", bufs=6) as mp,
            tc.tile_pool(name="xc", bufs=3) as xcp,
            tc.tile_pool(name="psb", bufs=3, space=bass.MemorySpace.PSUM) as psb,
            tc.tile_pool(name="ppp", bufs=3, space=bass.MemorySpace.PSUM) as ppp,
            tc.tile_pool(name="ptp", bufs=2, space=bass.MemorySpace.PSUM) as ptp,
        ):
            sA = main.tile([128, sec], dt.bfloat16)
            pself = main.tile([128, sec], dt.bfloat16)
            work = main.tile([128, 1, sec], dt.bfloat16)
            ident = main.tile([128, 128], dt.bfloat16)
            w1 = main.tile([128, p.finc, fmid], dt.bfloat16)
            w2 = main.tile([fmid, fmid], dt.bfloat16)
            w3 = main.tile([fmid, FOUT_PAD], dt.bfloat16)
            d1 = main.tile([128, nranks], dt.float32)
            d2 = main.tile([128, nranks], dt.float32)
            il = main.tile([128, ltot // 16], dt.int16)

            # load order follows first use: L1 dense needs ident/w1 (+xT,
            # issued in pstage_l1); idx/w2/w3/d's are needed only later
            nc.sync.dma_start(ident[:], id_d[:])
            nc.sync.dma_start(w1[:], w1_d[:])

            # rank groups for split allgather-input writes; group g (4 ranks)
            # is ready as soon as block g has been relu'd
            rgroups = [(4 * g, min(4 * g + 4, nranks), g) for g in range(nblk)]
            NG = len(rgroups)

            def ag_part(g):
                a, b, _ = rgroups[g]
                nc.sync.dma_start(ag_in[:, a * 128:b * 128],
                                  work[:, 0, a * 128:b * 128])

            def ag_finish(buf):
                if sim_mode:
                    # stand-in for the collective: tiny DMA bridging the
                    # ag_in -> AG dependency so the timeline stays ordered
                    nc.sync.dma_start(AG[buf][C * 64: C * 64 + 1, :], ag_in[0:1, :])
                else:
                    nc.gpsimd.collective_compute(
                        "AllGather", mybir.AluOpType.bypass,
                        replica_groups=[list(range(C))],
                        ins=[ag_in.ap().opt()], outs=[AG[buf].ap().opt()])

            def pself_group(a, b):
                # feature-major copy of ranks [a,b)'s fresh p-values (work is
                # token-major); used as the self-loop term of the next layer
                n = b - a
                pt = ptp.tile([128, 4, 128], dt.bfloat16, tag="pt")
                for i, c in enumerate(range(a, b)):
                    nc.tensor.transpose(pt[:, i, :],
                                        work[:, 0, c * 128:(c + 1) * 128],
                                        ident[:])
                nc.scalar.activation(
                    pself[:, a * 128:b * 128],
                    pt[:, 0:n, :].rearrange("q c f -> q (c f)"),
                    mybir.ActivationFunctionType.Copy)

            def pstage_group(a, b, w, scale):
                n = b - a
                ps = ppp.tile([128, 4, 128], dt.float32, tag="pp")
                for i, c in enumerate(range(a, b)):
                    nc.tensor.matmul(ps[:, i, 0:w.shape[-1]],
                                     work[:, 0, c * 128:(c + 1) * 128], w[:],
                                     start=True, stop=True)
                nc.vector.tensor_tensor(
                    work[:, 0, a * 128:b * 128].rearrange("q (c f) -> q c f", f=128),
                    ps[:, 0:n, :],
                    scale[:, a:b].rearrange("q c -> q c ()").broadcast_to(
                        [128, n, 128]),
                    mybir.AluOpType.mult)
                pself_group(a, b)

            def pstage_l1(buf):
                GRP = 13
                ngrp = (nranks + GRP - 1) // GRP
                xts = []
                for g in range(ngrp):
                    a, b = g * GRP, min(nranks, (g + 1) * GRP)
                    xt = mp.tile([128, b - a, p.finc, 128], dt.bfloat16, tag="x")
                    nc.sync.dma_start(xt[:], xT_d[:, a:b, :, :])
                    xts.append((a, xt))
                # remaining constants, after xT on the DMA queue
                nc.sync.dma_start(d1[:], d1_d[:])
                nc.sync.dma_start(d2[:], d2_d[:])
                nc.sync.dma_start(w2[:], w2_d[:])
                nc.sync.dma_start(w3[:], w3_d[:])
                nc.sync.dma_start(il[:], il_d[:])
                nc.vector.memset(sA[:], 0.0)
                nc.vector.memset(work[:], 0.0)
                for g in range(NG):
                    ga, gb, _ = rgroups[g]
                    ps = ppp.tile([128, 4, 128], dt.float32, tag="pp")
                    for i, c in enumerate(range(ga, gb)):
                        xa, xt = xts[c // GRP]
                        for f in range(p.finc):
                            nc.tensor.matmul(ps[:, i, :], xt[:, c - xa, f, :],
                                             w1[:, f, :],
                                             start=(f == 0), stop=(f == p.finc - 1))
                    nc.vector.tensor_copy(
                        work[:, 0, ga * 128:gb * 128].rearrange(
                            "q (c f) -> q c f", f=128),
                        ps[:, 0:gb - ga, :])
                    ag_part(g)
                    pself_group(ga, gb)
                ag_finish(buf)

            def out_group(a, b, sm, et, lg):
                n = b - a
                pt = ptp.tile([128, 4, 128], dt.bfloat16, tag="pt")
                for i, c in enumerate(range(a, b)):
                    nc.tensor.transpose(pt[:, i, :], sA[:, c * 128:(c + 1) * 128],
                                        ident[:])
                nc.vector.tensor_tensor(
                    sm[:, a:b, :], pt[:, 0:n, 0:fout],
                    d1[:, a:b].rearrange("q c -> q c ()").broadcast_to(
                        [128, n, fout]),
                    mybir.AluOpType.mult)
                nc.scalar.activation(et[:, a:b, :], sm[:, a:b, :],
                                     mybir.ActivationFunctionType.Exp)
                nc.vector.reduce_sum(lg[:, a:b], et[:, a:b, :],
                                     axis=mybir.AxisListType.X)

            def aggregate(layer):
                """Gather + identity-matmul segment-sum; per block: drain into
                sA, then immediately run whatever downstream work that block
                unblocks (relu + dense + ag write, or the output ranks)."""
                buf = layer % 2
                in_ap = AG[buf].ap().rearrange(
                    "a (r f) -> (a r) f", f=128)[p.half:, :]
                mts = {}
                for ci, clen in enumerate(p.chunks):
                    m = mp.tile([128, 1, clen], dt.bfloat16, tag="m")
                    nc.gpsimd.dma_gather(
                        out_ap=m[:], in_ap=in_ap,
                        idxs_ap=il[:, coff[ci] // 16:(coff[ci] + clen) // 16],
                        num_idxs=clen, num_idxs_reg=clen, elem_size=128,
                        transpose=True, single_packet=False, queue_num=0)
                    mts[ci] = m
                last = layer == 2
                if last:
                    sm = mp.tile([128, nranks, fout], dt.float32, tag="o")
                    et = mp.tile([128, nranks, fout], dt.float32, tag="o")
                    lg = xcp.tile([128, nranks], dt.float32, tag="xc")
                    w_next = scale = None
                else:
                    w_next = w2 if layer == 0 else w3
                    scale = d2
                for b in p.border:
                    bsz = min(BLK, npc - b * BLK)
                    a0 = b * BLK
                    ps = psb.tile([128, BLK], dt.float32, tag="ps")
                    # self-loop term opens the accumulation group
                    nc.tensor.matmul(ps[:, 0:bsz], ident[:], pself[:, a0:a0 + bsz],
                                     start=True, stop=False)
                    plist = [(k, pc) for k in range(kmax[b] + 1)
                             for pc in p.pieces[(b, k)]]
                    for i, (k, (ci, off, t, col0)) in enumerate(plist):
                        nc.tensor.matmul(ps[:, col0:col0 + t], ident[:],
                                         mts[ci][:, 0, off:off + t],
                                         start=False, stop=(i == len(plist) - 1))
                    if last:
                        # out-stage transposes need SBUF input: drain to sA
                        nc.vector.tensor_copy(sA[:, a0:a0 + bsz], ps[:, 0:bsz])
                        out_group(4 * b, min(4 * b + 4, nranks), sm, et, lg)
                    else:
                        # relu straight from PSUM; sA is not needed
                        nc.scalar.activation(work[:, 0, a0:a0 + bsz], ps[:, 0:bsz],
                                             mybir.ActivationFunctionType.Relu)
                        pstage_group(rgroups[b][0], rgroups[b][1], w_next, scale)
                        ag_part(b)
                if last:
                    nc.scalar.activation(lg[:], lg[:],
                                         mybir.ActivationFunctionType.Ln)
                    qs = [0, 12, 24, 36, nranks]
                    for i in range(4):
                        a, b = qs[i], qs[i + 1]
                        nc.vector.tensor_tensor(
                            sm[:, a:b, :], sm[:, a:b, :],
                            lg[:, a:b].rearrange("q c -> q c ()").broadcast_to(
                                [128, b - a, fout]),
                            mybir.AluOpType.subtract)
                        nc.sync.dma_start(
                            out_d[:, a * fout:b * fout],
                            sm[:, a:b, :].rearrange("q c f -> q (c f)"))
                else:
                    ag_finish((layer + 1) % 2)

            # ---- program ----
            pstage_l1(0)
            for layer in range(3):
                aggregate(layer)
    nc.compile()
    return nc


# --------------------------------------------------------------------------
# Entry point
# --------------------------------------------------------------------------
def _make_in_maps(p, inputs, fmid, fout):
    W_in = np.asarray(inputs["W_in"], dtype=np.float32)
    W_mid = np.asarray(inputs["W_mid"], dtype=np.float32)
    W_out = np.asarray(inputs["W_out"], dtype=np.float32)
    w1 = np.ascontiguousarray(
        W_in.reshape(p.finc, 128, fmid).transpose(1, 0, 2).astype(BF16))
    w2 = np.ascontiguousarray(W_mid.astype(BF16))
    w3 = np.zeros((fmid, FOUT_PAD), BF16)
    w3[:, :fout] = W_out.astype(BF16)
    ident = np.eye(128, dtype=np.float32).astype(BF16)
    in_maps = []
    for m in range(C):
        in_maps.append({
            "xT": p.xT[m].reshape(128, p.nranks, p.finc, 128),
            "w1": w1, "w2": w2, "w3": w3,
            "d1": p.d1[m], "d2": p.d2[m], "ident": ident,
            "idx": p.idx[m],
        })
    return in_maps


def _run(inputs, trace=False, trace_cores=None, debug_stage=99):
    x = np.asarray(inputs["x"], dtype=np.float32)
    edge_index = np.asarray(inputs["edge_index"])
    W_in = np.asarray(inputs["W_in"], dtype=np.float32)
    W_out = np.asarray(inputs["W_out"], dtype=np.float32)
    for bname in ("b_in", "b_mid", "b_out"):
        if np.any(np.asarray(inputs[bname])):
            raise NotImplementedError("nonzero bias path not implemented")

    N, fin = x.shape
    fmid = W_in.shape[1]
    fout = W_out.shape[1]
    p = preprocess(x, edge_index, N)

    nc = build_kernel(p, fmid, fout)

    in_maps = _make_in_maps(p, inputs, fmid, fout)
    kw = {}
    if trace:
        kw = dict(trace=True, trace_cores=trace_cores or [0])
    r = run_bass_kernel_spmd(nc, in_maps, core_ids=list(range(C)), **kw)

    out = np.empty((N, fout), np.float32)
    for m in range(C):
        res = r.results[m]["out"]  # [128, nranks*fout] partition-major
        rows = res.reshape(128, p.nranks, fout).transpose(1, 0, 2).reshape(p.sec, fout)
        out[m * p.npc + p.perm[m]] = rows[:p.npc]
    return out, r


def kernel(**inputs) -> np.ndarray:
    out, _ = _run(inputs)
    return out


# revision 45
# speedup vs baseline: 1.0105x; 1.0036x over previous
"""3-layer GCN (GCNConv x3, PyG defaults) on 8 Trainium2 NeuronCores.

Strategy (graph/data parallel, v2):
  - Nodes sharded 8 ways by destination range (6250/core, padded to 6272).
    Per core, nodes are sorted by total in-degree (incl. self-loop) desc;
    slot k of the segment-sum covers the prefix of nodes with degree > k.
  - Per layer: dense X@W on the TensorEngine (activations kept as
    [token, feat] per 128-token rank), scaled by deg^-1/2 into p = dinv*h,
    one AllGather collective puts every core's p into a DRAM buffer AG
    laid out as 50176 contiguous 256 B tokens.
  - Aggregation gathers each edge's source token DIRECTLY FROM DRAM with
    gpsimd dma_gather (transposed output [128 feat, edges]) and segment-sums
    on the TensorEngine as identity-matmul PSUM accumulation, slot-major.
    A SINGLE pass covers all 50176 source tokens: int16 gather indices are
    signed, so the gather base is placed at token 25088 and indices span
    [-25088, 25087]. The SWDGE ucode trims trailing NEGATIVE indices, so
    every gather chunk is padded with a positive zero-data token.
  - All deg^-1/2 scales fold into per-partition scalars of PSUM-drain
    activations (relu(d*x) = d*relu(x) for d>0).
All 8 cores run one identical program; only input data differs per core.
"""
import sys

sys.path.insert(0, "/opt/trn_rl_repo")

import numpy as np
import ml_dtypes

from concourse import bass, bacc, mybir
from concourse import tile
from concourse.bass_utils import run_bass_kernel_spmd

BF16 = ml_dtypes.bfloat16
C = 8
BLK = 512
CHUNK = 4096
FOUT_PAD = 128  # W_out columns padded so layer-3 tokens share the 256B layout


class Plan:
    pass


def _pack_idx(vals):
    """int16 array -> [128, len/16] wrapped (i -> [i%16, i//16]) replicated x8."""
    n = len(vals)
    assert n % 16 == 0
    a = np.asarray(vals, dtype=np.int16).reshape(n // 16, 16).T  # [16, n/16]
    return np.tile(a, (8, 1))


def preprocess(x, edge_index, n_nodes):
    p = Plan()
    N = n_nodes
    assert N % C == 0
    npc = N // C
    sec = (npc + 127) // 128 * 128
    p.npc, p.sec = npc, sec
    p.nranks = sec // 128
    p.nblk = (npc + BLK - 1) // BLK
    p.fin = x.shape[1]
    assert p.fin % 128 == 0
    p.finc = p.fin // 128
    ntok = C * sec
    half = ntok // 2
    p.half = half
    # positive zero-data pad token: last pad position of core C-1
    pad_pos = sec - 1
    PADTOK = ((C - 1) * 128 + pad_pos % 128) * p.nranks + pad_pos // 128
    assert PADTOK >= half

    src = np.asarray(edge_index[0], dtype=np.int64)
    dst = np.asarray(edge_index[1], dtype=np.int64)
    indeg = np.bincount(dst, minlength=N).astype(np.int64)  # real in-edges
    dinv = (1.0 / np.sqrt((indeg + 1).astype(np.float32))).astype(np.float32)

    # per-core node order: in-degree desc; position within core
    pos_g = np.empty(N, np.int64)
    perm = np.empty((C, npc), np.int64)
    for m in range(C):
        pl = np.argsort(-indeg[m * npc:(m + 1) * npc], kind="stable")
        perm[m] = pl
        pos_g[m * npc + pl] = np.arange(npc)
    p.perm = perm
    # DRAM token id of each node: t = (core*128 + pos%128)*nranks + pos//128
    tok = ((np.arange(N) // npc) * 128 + pos_g % 128) * p.nranks + pos_g // 128

    # slot-major cell structure over real edges (self term is added on-chip)
    d_core = dst // npc
    all_cells = [None] * C
    all_data = [None] * C
    for m in range(C):
        sel = d_core == m
        jp = pos_g[dst[sel]]
        ts = tok[src[sel]]
        cnt = np.bincount(jp, minlength=npc)
        order = np.argsort(jp, kind="stable")
        js, tss = jp[order], ts[order]
        starts = np.zeros(npc, np.int64)
        starts[1:] = np.cumsum(cnt)[:-1]
        kk = np.arange(len(js)) - starts[js]
        bb = js // BLK
        o2 = np.lexsort((js, kk, bb))
        js, tss, kk, bb = js[o2], tss[o2], kk[o2], bb[o2]
        cells = {}
        data = {}
        cell_id = bb * 8192 + kk
        uniq, first = np.unique(cell_id, return_index=True)
        bounds = list(first) + [len(cell_id)]
        for i, u in enumerate(uniq):
            b, k = int(u) // 8192, int(u) % 8192
            seg = slice(bounds[i], bounds[i + 1])
            cells[(b, k)] = bounds[i + 1] - bounds[i]
            data[(b, k)] = tss[seg]
        all_cells[m] = cells
        all_data[m] = data

    # uniform geometry across cores: per-cell max width and column offset
    cells_max = {}
    for m in range(C):
        for key, n in all_cells[m].items():
            cells_max[key] = max(cells_max.get(key, 0), n)
    cell_col = {}  # (b,k) -> first covered PSUM column (cells cover a prefix)
    for (b, k), n in cells_max.items():
        cell_col[(b, k)] = 0

    # pack cells (b-major, k asc) into chunks; cells may split across chunks;
    # every chunk reserves >=1 trailing positive pad column (negative-trim
    # workaround), and lengths are multiples of 128
    # block processing order: heavy late blocks first, the tiny last block
    # (106 cols, single-rank dense group) at the very end so the per-layer
    # tail (last block -> dense -> allgather write) is minimal
    p.border = [9, 10, 11] + list(range(9)) + [p.nblk - 1]
    chunks = []
    pieces = {}  # (b,k) -> list of (chunk_idx, off_in_chunk, ncols, col0)
    cur = CHUNK  # force new chunk
    for b in p.border:
        ks = sorted(k for (bb2, k) in cells_max if bb2 == b)
        for k in ks:
            n = cells_max[(b, k)]
            col0 = 0
            pl = []
            while n > 0:
                # first two chunks are half-size: their desc-gen pipelines
                # right after the collective, shortening each layer's ramp
                cap = 2048 if len(chunks) <= 2 else CHUNK
                if cur >= cap - 1:
                    chunks.append(0)
                    cur = 0
                    cap = 2048 if len(chunks) <= 2 else CHUNK
                t = min(n, cap - 1 - cur)
                pl.append((len(chunks) - 1, cur, t, col0))
                cur += t
                chunks[-1] = cur
                col0 += t
                n -= t
            pieces[(b, k)] = pl
    chunks = [l + 128 - l % 128 if l % 128 else l + 128 for l in chunks]
    p.chunks = chunks
    p.pieces = pieces
    total = sum(chunks)
    coff = np.concatenate([[0], np.cumsum(chunks)])

    p.idx = []
    for m in range(C):
        buf = np.full(total, PADTOK, np.int64)
        for key, pl in pieces.items():
            d = all_data[m].get(key)
            if d is None:
                continue
            for (ci, off, t, col0) in pl:
                seg = d[col0:col0 + t]
                buf[coff[ci] + off: coff[ci] + off + len(seg)] = seg
        p.idx.append(_pack_idx((buf - half).astype(np.int16)))

    # per-core dense inputs; d1 (= deg^-1/2 per token) is folded into x so the
    # layer-1 dense matmul directly produces p1 = d1 * (x @ W1)
    p.xT = []
    p.d1 = []
    p.d2 = []
    for m in range(C):
        pl = perm[m]
        xm = np.asarray(x[m * npc:(m + 1) * npc][pl], dtype=np.float32)
        xm = xm * dinv[m * npc + pl][:, None]
        xt = np.zeros((p.nranks, 128, p.finc, 128), BF16)
        flat = xt.reshape(sec, p.finc, 128)
        flat[:npc] = xm.reshape(npc, p.finc, 128).astype(BF16)
        p.xT.append(np.ascontiguousarray(xt.transpose(1, 0, 2, 3)))
        dv = np.zeros(sec, np.float32)
        dv[:npc] = dinv[m * npc + pl]
        dvt = dv.reshape(p.nranks, 128).T.copy()  # [128, nranks]
        p.d1.append(dvt)
        p.d2.append(dvt * dvt)
    return p


# --------------------------------------------------------------------------
# Device kernel builder (one program, SPMD across 8 cores).
# --------------------------------------------------------------------------
def build_kernel(p, fmid, fout, bias_path=False, debug_stage=99, sim_mode=False):
    dt = mybir.dt
    nc = bacc.Bacc("TRN2", num_swdge_queues=1)
    sec, nranks, npc, nblk = p.sec, p.nranks, p.npc, p.nblk
    ntok = C * sec
    ltot = sum(p.chunks)

    xT_d = nc.dram_tensor("xT", [128, nranks, p.finc, 128], dt.bfloat16, kind="ExternalInput")
    w1_d = nc.dram_tensor("w1", [128, p.finc, fmid], dt.bfloat16, kind="ExternalInput")
    w2_d = nc.dram_tensor("w2", [fmid, fmid], dt.bfloat16, kind="ExternalInput")
    w3_d = nc.dram_tensor("w3", [fmid, FOUT_PAD], dt.bfloat16, kind="ExternalInput")
    d1_d = nc.dram_tensor("d1", [128, nranks], dt.float32, kind="ExternalInput")
    d2_d = nc.dram_tensor("d2", [128, nranks], dt.float32, kind="ExternalInput")
    id_d = nc.dram_tensor("ident", [128, 128], dt.bfloat16, kind="ExternalInput")
    il_d = nc.dram_tensor("idx", [128, ltot // 16], dt.int16, kind="ExternalInput")
    out_d = nc.dram_tensor("out", [128, nranks * fout], dt.float32, kind="ExternalOutput")

    ag_in = nc.dram_tensor("ag_in", [128, sec], dt.bfloat16)
    AG = [nc.dram_tensor(f"ag_out{i}", [C * 128, sec], dt.bfloat16, addr_space="Shared")
          for i in range(2)]

    # kmax per block
    kmax = {}
    for (b, k) in p.pieces:
        kmax[b] = max(kmax.get(b, -1), k)
    coff = [0]
    for l in p.chunks:
        coff.append(coff[-1] + l)

    with tile.TileContext(nc) as tc:
        with (
            tc.tile_pool(name="main", bufs=1) as main,
            tc.tile_pool(name="mp", bufs=6) as mp,
            tc.tile_pool(name="xc", bufs=3) as xcp,
            tc.tile_pool(name="psb", bufs=3, space=bass.MemorySpace.PSUM) as psb,
            tc.tile_pool(name="ppp", bufs=3, space=bass.MemorySpace.PSUM) as ppp,
            tc.tile_pool(name="ptp", bufs=2, space=bass.MemorySpace.PSUM) as ptp,
        ):
            sA = main.tile([128, sec], dt.bfloat16)
            pself = main.tile([128, sec], dt.bfloat16)
            work = main.tile([128, 1, sec], dt.bfloat16)
            ident = main.tile([128, 128], dt.bfloat16)
            w1 = main.tile([128, p.finc, fmid], dt.bfloat16)
            w2 = main.tile([fmid, fmid], dt.bfloat16)
            w3 = main.tile([fmid, FOUT_PAD], dt.bfloat16)
            d1 = main.tile([128, nranks], dt.float32)
            d2 = main.tile([128, nranks], dt.float32)
            il = main.tile([128, ltot // 16], dt.int16)

            # load order follows first use: L1 dense needs ident/w1 (+xT,
            # issued in pstage_l1); idx/w2/w3/d's are needed only later
            nc.sync.dma_start(ident[:], id_d[:])
            nc.sync.dma_start(w1[:], w1_d[:])

            # rank groups for split allgather-input writes; group g (4 ranks)
            # is ready as soon as block g has been relu'd
            rgroups = [(4 * g, min(4 * g + 4, nranks), g) for g in range(nblk)]
            NG = len(rgroups)

            def ag_part(g):
                a, b, _ = rgroups[g]
                nc.sync.dma_start(ag_in[:, a * 128:b * 128],
                                  work[:, 0, a * 128:b * 128])

            def ag_finish(buf):
                if sim_mode:
                    # stand-in for the collective: tiny DMA bridging the
                    # ag_in -> AG dependency so the timeline stays ordered;
                    # issued from the Act queue so its launch overlaps the
                    # SP-queue allgather-part launches
                    nc.scalar.dma_start(AG[buf][C * 64: C * 64 + 1, :], ag_in[0:1, :])
                else:
                    nc.gpsimd.collective_compute(
                        "AllGather", mybir.AluOpType.bypass,
                        replica_groups=[list(range(C))],
                        ins=[ag_in.ap().opt()], outs=[AG[buf].ap().opt()])

            def pself_group(a, b):
                # feature-major copy of ranks [a,b)'s fresh p-values (work is
                # token-major); used as the self-loop term of the next layer
                n = b - a
                pt = ptp.tile([128, 4, 128], dt.bfloat16, tag="pt")
                for i, c in enumerate(range(a, b)):
                    nc.tensor.transpose(pt[:, i, :],
                                        work[:, 0, c * 128:(c + 1) * 128],
                                        ident[:])
                nc.scalar.activation(
                    pself[:, a * 128:b * 128],
                    pt[:, 0:n, :].rearrange("q c f -> q (c f)"),
                    mybir.ActivationFunctionType.Copy)

            def pstage_group(a, b, w, scale):
                n = b - a
                ps = ppp.tile([128, 4, 128], dt.float32, tag="pp")
                for i, c in enumerate(range(a, b)):
                    nc.tensor.matmul(ps[:, i, 0:w.shape[-1]],
                                     work[:, 0, c * 128:(c + 1) * 128], w[:],
                                     start=True, stop=True)
                nc.vector.tensor_tensor(
                    work[:, 0, a * 128:b * 128].rearrange("q (c f) -> q c f", f=128),
                    ps[:, 0:n, :],
                    scale[:, a:b].rearrange("q c -> q c ()").broadcast_to(
                        [128, n, 128]),
                    mybir.AluOpType.mult)
                pself_group(a, b)

            def pstage_l1(buf):
                GRP = 13
                ngrp = (nranks + GRP - 1) // GRP
                xts = []
                for g in range(ngrp):
                    a, b = g * GRP, min(nranks, (g + 1) * GRP)
                    xt = mp.tile([128, b - a, p.finc, 128], dt.bfloat16, tag="x")
                    nc.sync.dma_start(xt[:], xT_d[:, a:b, :, :])
                    xts.append((a, xt))
                # remaining constants, after xT on the DMA queue
                nc.sync.dma_start(d1[:], d1_d[:])
                nc.sync.dma_start(d2[:], d2_d[:])
                nc.sync.dma_start(w2[:], w2_d[:])
                nc.sync.dma_start(w3[:], w3_d[:])
                nc.sync.dma_start(il[:], il_d[:])
                nc.vector.memset(sA[:], 0.0)
                nc.vector.memset(work[:], 0.0)
                for g in range(NG):
                    ga, gb, _ = rgroups[g]
                    ps = ppp.tile([128, 4, 128], dt.float32, tag="pp")
                    for i, c in enumerate(range(ga, gb)):
                        xa, xt = xts[c // GRP]
                        for f in range(p.finc):
                            nc.tensor.matmul(ps[:, i, :], xt[:, c - xa, f, :],
                                             w1[:, f, :],
                                             start=(f == 0), stop=(f == p.finc - 1))
                    nc.vector.tensor_copy(
                        work[:, 0, ga * 128:gb * 128].rearrange(
                            "q (c f) -> q c f", f=128),
                        ps[:, 0:gb - ga, :])
                    ag_part(g)
                    pself_group(ga, gb)
                ag_finish(buf)

            def out_group(a, b, sm, et, lg):
                n = b - a
                pt = ptp.tile([128, 4, 128], dt.bfloat16, tag="pt")
                for i, c in enumerate(range(a, b)):
                    nc.tensor.transpose(pt[:, i, :], sA[:, c * 128:(c + 1) * 128],
                                        ident[:])
                nc.vector.tensor_tensor(
                    sm[:, a:b, :], pt[:, 0:n, 0:fout],
                    d1[:, a:b].rearrange("q c -> q c ()").broadcast_to(
                        [128, n, fout]),
                    mybir.AluOpType.mult)
                nc.scalar.activation(et[:, a:b, :], sm[:, a:b, :],
                                     mybir.ActivationFunctionType.Exp)
                nc.vector.reduce_sum(lg[:, a:b], et[:, a:b, :],
                                     axis=mybir.AxisListType.X)

            def aggregate(layer):
                """Gather + identity-matmul segment-sum; per block: drain into
                sA, then immediately run whatever downstream work that block
                unblocks (relu + dense + ag write, or the output ranks)."""
                buf = layer % 2
                in_ap = AG[buf].ap().rearrange(
                    "a (r f) -> (a r) f", f=128)[p.half:, :]
                mts = {}
                for ci, clen in enumerate(p.chunks):
                    m = mp.tile([128, 1, clen], dt.bfloat16, tag="m")
                    nc.gpsimd.dma_gather(
                        out_ap=m[:], in_ap=in_ap,
                        idxs_ap=il[:, coff[ci] // 16:(coff[ci] + clen) // 16],
                        num_idxs=clen, num_idxs_reg=clen, elem_size=128,
                        transpose=True, single_packet=False, queue_num=0)
                    mts[ci] = m
                last = layer == 2
                if last:
                    sm = mp.tile([128, nranks, fout], dt.float32, tag="o")
                    et = mp.tile([128, nranks, fout], dt.float32, tag="o")
                    lg = xcp.tile([128, nranks], dt.float32, tag="xc")
                    # dummy ln: pulls the natural-log act-table load off the
                    # final-output critical path (harmless value, overwritten)
                    nc.vector.memset(lg[:, 0:1], 1.0)
                    nc.scalar.activation(lg[:, 0:1], lg[:, 0:1],
                                         mybir.ActivationFunctionType.Ln)
                    w_next = scale = None
                else:
                    w_next = w2 if layer == 0 else w3
                    scale = d2
                for b in p.border:
                    bsz = min(BLK, npc - b * BLK)
                    a0 = b * BLK
                    ps = psb.tile([128, BLK], dt.float32, tag="ps")
                    # self-loop term opens the accumulation group
                    nc.tensor.matmul(ps[:, 0:bsz], ident[:], pself[:, a0:a0 + bsz],
                                     start=True, stop=False)
                    plist = [(k, pc) for k in range(kmax[b] + 1)
                             for pc in p.pieces[(b, k)]]
                    for i, (k, (ci, off, t, col0)) in enumerate(plist):
                        nc.tensor.matmul(ps[:, col0:col0 + t], ident[:],
                                         mts[ci][:, 0, off:off + t],
                                         start=False, stop=(i == len(plist) - 1))
                    if last:
                        # out-stage transposes need SBUF input: drain to sA
                        nc.vector.tensor_copy(sA[:, a0:a0 + bsz], ps[:, 0:bsz])
                        out_group(4 * b, min(4 * b + 4, nranks), sm, et, lg)
                    else:
                        # relu straight from PSUM; sA is not needed
                        nc.scalar.activation(work[:, 0, a0:a0 + bsz], ps[:, 0:bsz],
                                             mybir.ActivationFunctionType.Relu)
                        pstage_group(rgroups[b][0], rgroups[b][1], w_next, scale)
                        ag_part(b)
                if last:
                    nc.scalar.activation(lg[:], lg[:],
                                         mybir.ActivationFunctionType.Ln)
                    qs = [0, 12, 24, 36, nranks]
                    for i in range(4):
                        a, b = qs[i], qs[i + 1]
                        nc.vector.tensor_tensor(
                            sm[:, a:b, :], sm[:, a:b, :],
                            lg[:, a:b].rearrange("q c -> q c ()").broadcast_to(
                                [128, b - a, fout]),
                            mybir.AluOpType.subtract)
                        nc.sync.dma_start(
                            out_d[:, a * fout:b * fout],
                            sm[:, a:b, :].rearrange("q c f -> q (c f)"))
                else:
                    ag_finish((layer + 1) % 2)

            # ---- program ----
            pstage_l1(0)
            for layer in range(3):
                aggregate(layer)
    nc.compile()
    return nc


# --------------------------------------------------------------------------
# Entry point
# --------------------------------------------------------------------------
def _make_in_maps(p, inputs, fmid, fout):
    W_in = np.asarray(inputs["W_in"], dtype=np.float32)
    W_mid = np.asarray(inputs["W_mid"], dtype=np.float32)
    W_out = np.asarray(inputs["W_out"], dtype=np.float32)
    w1 = np.ascontiguousarray(
        W_in.reshape(p.finc, 128, fmid).transpose(1, 0, 2).astype(BF16))
    w2 = np.ascontiguousarray(W_mid.astype(BF16))
    w3 = np.zeros((fmid, FOUT_PAD), BF16)
    w3[:, :fout] = W_out.astype(BF16)
    ident = np.eye(128, dtype=np.float32).astype(BF16)
    in_maps = []
    for m in range(C):
        in_maps.append({
            "xT": p.xT[m].reshape(128, p.nranks, p.finc, 128),
            "w1": w1, "w2": w2, "w3": w3,
            "d1": p.d1[m], "d2": p.d2[m], "ident": ident,
            "idx": p.idx[m],
        })
    return in_maps


def _run(inputs, trace=False, trace_cores=None, debug_stage=99):
    x = np.asarray(inputs["x"], dtype=np.float32)
    edge_index = np.asarray(inputs["edge_index"])
    W_in = np.asarray(inputs["W_in"], dtype=np.float32)
    W_out = np.asarray(inputs["W_out"], dtype=np.float32)
    for bname in ("b_in", "b_mid", "b_out"):
        if np.any(np.asarray(inputs[bname])):
            raise NotImplementedError("nonzero bias path not implemented")

    N, fin = x.shape
    fmid = W_in.shape[1]
    fout = W_out.shape[1]
    p = preprocess(x, edge_index, N)

    nc = build_kernel(p, fmid, fout)

    in_maps = _make_in_maps(p, inputs, fmid, fout)
    kw = {}
    if trace:
        kw = dict(trace=True, trace_cores=trace_cores or [0])
    r = run_bass_kernel_spmd(nc, in_maps, core_ids=list(range(C)), **kw)

    out = np.empty((N, fout), np.float32)
    for m in range(C):
        res = r.results[m]["out"]  # [128, nranks*fout] partition-major
        rows = res.reshape(128, p.nranks, fout).transpose(1, 0, 2).reshape(p.sec, fout)
        out[m * p.npc + p.perm[m]] = rows[:p.npc]
    return out, r


def kernel(**inputs) -> np.ndarray:
    out, _ = _run(inputs)
    return out
